# revision 1
# baseline (speedup 1.0000x reference)
"""CompressedLinear Trainium2 kernel.

Computes out[b,s,o] = x[b,s,i] @ (int8_weight[o,i] * scale).T + bias[o]
with x: [4,2048,4096] f32, weight_int8: [11008,4096] int32 (int8 values),
scale: scalar f32, bias: [11008] f32.

Sharding: column-parallel over 8 NeuronCores — each core owns 1376
out-features (weight + bias slice), x is replicated, outputs concat on
the last dim.

Per-core device kernel (Bass/Tile):
  - weight slice is uploaded in [in, out] layout in its compressed int8
    form; the device dequantizes shard-locally: SWDGE cast-DMA
    int8 -> bf16 (exact for int8-range values) into resident SBUF tiles
    totalling [4096 x 1376].
  - x is uploaded in [in, s] layout (f32); streamed as SWDGE cast-DMA
    f32 -> bf16 tiles.
  - TensorE: psum[s=128, o<=512] += xT_tile[k,s].T-free @ wT_tile[k,o]
    accumulated over 32 k-tiles of 128.
  - epilogue (DVE): out = psum * scale + bias in one scalar_tensor_tensor,
    then HWDGE store to DRAM in natural [s, o] layout.
"""

import numpy as np

import concourse.bacc as bacc
import concourse.mybir as mybir
import concourse.tile as tile
from concourse.bass_utils import run_bass_kernel_spmd

# Problem shape (hardcoded per contract)
B, S, IN_F, OUT_F = 4, 2048, 4096, 11008
NCORES = 8
OUT_PER = OUT_F // NCORES  # 1376
S_TOT = B * S  # 8192

# Tiling
KTILE = 128  # contraction per matmul
S_CHUNK = 512  # s-columns per x-load group
S_SUB = 128  # out-rows per psum block
KGRP = 4  # k-tiles per x DMA (1 MiB f32 reads)
NMAX = 512  # max moving free dim / psum bank

# set by test harness to capture profiles; harness calls kernel() untouched
TRACE = False
LAST_RESULT = None

_cache = {}


def _n_chunks(out_per):
    chunks = []
    off = 0
    while off < out_per:
        sz = min(NMAX, out_per - off)
        chunks.append((off, sz))
        off += sz
    return chunks


def build_nc(s_tot=S_TOT, in_f=IN_F, out_per=OUT_PER, s_chunk=S_CHUNK, kgrp=KGRP):
    f32 = mybir.dt.float32
    bf16 = mybir.dt.bfloat16
    i8 = mybir.dt.int8

    KT = in_f // KTILE  # k-tiles
    NKG = KT // kgrp  # x-load groups per s-chunk
    chunks = _n_chunks(out_per)

    nc = bacc.Bacc("TRN2", target_bir_lowering=False, debug=False, num_devices=NCORES)

    xt = nc.dram_tensor("xt", [in_f, s_tot], f32, kind="ExternalInput").ap()
    wt = nc.dram_tensor("wt", [in_f, out_per], i8, kind="ExternalInput").ap()
    bias = nc.dram_tensor("bias", [1, out_per], f32, kind="ExternalInput").ap()
    scale = nc.dram_tensor("scale", [1, 1], f32, kind="ExternalInput").ap()
    out = nc.dram_tensor("out", [s_tot, out_per], f32, kind="ExternalOutput").ap()

    # s-chunk schedule: narrow warmup chunks so the first psum blocks aren't
    # gated on the full 8 MB x-chunk + 5.6 MB weight load.
    warm = min(s_chunk // 2, 256)
    if s_tot > 2 * warm and (s_tot - 2 * warm) % s_chunk == 0:
        chunk_sched = [warm, warm] + [s_chunk] * ((s_tot - 2 * warm) // s_chunk)
    else:
        chunk_sched = [s_chunk] * (s_tot // s_chunk)

    with tile.TileContext(nc) as tc:
        with (
            tc.tile_pool(name="wt", bufs=1) as wt_pool,
            tc.tile_pool(name="xbf", bufs=2 * NKG + 3) as xbf_pool,
            tc.tile_pool(name="psum", bufs=2, space="PSUM") as psum_pool,
            tc.tile_pool(name="osb", bufs=4) as osb_pool,
            tc.tile_pool(name="consts", bufs=1) as const_pool,
        ):
            # HAM warmup: dummy matmuls on zeroed SBUF while the first loads
            # are in flight, so the PE clock-gate (4/8 cold -> 8/8 warm after
            # ~3.4us of activity) opens before real matmuls start.
            zeros = const_pool.tile([128, NMAX], bf16, tag="zeros", name="zeros")
            nc.vector.memset(zeros[:], 0)
            psw = psum_pool.tile([128, NMAX], f32, tag="warm", name="warm", bufs=1)
            # 16 full-width MMs trip the activity window, then narrow (56ns)
            # ones keep the PE busy until the first loads land, whenever this
            # build's schedule makes that happen (14.5-17.5us observed) —
            # an idle >3.4us would re-throttle the clock to 4/8.
            for i in range(16):
                nc.tensor.matmul(
                    psw[:, :], zeros[:, 0:128], zeros[:, :], start=True, stop=True
                )
            for i in range(44):
                nc.tensor.matmul(
                    psw[:, 0:128],
                    zeros[:, 0:128],
                    zeros[:, 0:128],
                    start=True,
                    stop=True,
                )

            # Startup: interleave weight dequant (int8 -> bf16 cast DMA, exact
            # for int8-range values) with the first s-chunk's x loads, x tile
            # first — the tensor engine needs (xg0, wtg0) for its first MM.
            # The very first (x, w) pair covers a single k-tile so the first
            # matmul's dependencies are a few hundred KB, not MBs.
            groups0 = [(0, 1), (1, kgrp - 1)] + [
                (g * kgrp, kgrp) for g in range(1, NKG)
            ]
            sc0 = chunk_sched[0]
            wtk = {}  # k -> (tile, idx within tile)
            xg0 = {}
            for gi, (k0, kn) in enumerate(groups0):
                t = xbf_pool.tile([128, kn, sc0], bf16, tag="xbf", name=f"x0_{gi}")
                src = xt[k0 * 128 : (k0 + kn) * 128, 0:sc0].rearrange(
                    "(g p) s -> p g s", p=128
                )
                nc.gpsimd.dma_start(out=t[:], in_=src)
                for i in range(kn):
                    xg0[k0 + i] = (t, i)
                wtile = wt_pool.tile(
                    [128, kn, out_per], bf16, tag=f"wt{gi}", name=f"wt{gi}"
                )
                wsrc = wt[k0 * 128 : (k0 + kn) * 128, :].rearrange(
                    "(g p) o -> p g o", p=128
                )
                nc.gpsimd.dma_start(out=wtile[:], in_=wsrc)
                for i in range(kn):
                    wtk[k0 + i] = (wtile, i)

            scale_sb = const_pool.tile([128, 1], f32, tag="scale", name="scale_sb")
            nc.sync.dma_start(out=scale_sb[:], in_=scale.partition_broadcast(128))
            bias_sb = const_pool.tile([128, out_per], f32, tag="bias", name="bias_sb")
            nc.sync.dma_start(out=bias_sb[:], in_=bias.partition_broadcast(128))

            s0 = 0
            for ci, sc in enumerate(chunk_sched):
                if ci == 0:
                    xg = xg0
                else:
                    # x chunk load: cast f32 -> bf16 in DMA, [128, kgrp, sc]
                    xg = {}
                    for g in range(NKG):
                        t = xbf_pool.tile(
                            [128, kgrp, sc], bf16, tag="xbf", name=f"x{ci}_{g}"
                        )
                        src = xt[
                            g * kgrp * 128 : (g + 1) * kgrp * 128, s0 : s0 + sc
                        ].rearrange("(g p) s -> p g s", p=128)
                        nc.gpsimd.dma_start(out=t[:], in_=src)
                        for i in range(kgrp):
                            xg[g * kgrp + i] = (t, i)

                for sub in range(sc // S_SUB):
                    psums = [
                        psum_pool.tile(
                            [128, NMAX], f32, tag=f"ps{j}", name=f"ps{ci}_{sub}_{j}"
                        )
                        for j in range(len(chunks))
                    ]
                    for k in range(KT):
                        xt_t, xi = xg[k]
                        w_t, wi = wtk[k]
                        lhsT = xt_t[:, xi, sub * 128 : (sub + 1) * 128]
                        for j, (off, sz) in enumerate(chunks):
                            nc.tensor.matmul(
                                psums[j][:, :sz],
                                lhsT,
                                w_t[:, wi, off : off + sz],
                                start=(k == 0),
                                stop=(k == KT - 1),
                            )
                    osb = osb_pool.tile(
                        [128, out_per], f32, tag="osb", name=f"o{ci}_{sub}"
                    )
                    r0 = s0 + sub * S_SUB
                    for j, (off, sz) in enumerate(chunks):
                        nc.vector.scalar_tensor_tensor(
                            osb[:, off : off + sz],
                            psums[j][:, :sz],
                            scale_sb[:, 0:1],
                            bias_sb[:, off : off + sz],
                            mybir.AluOpType.mult,
                            mybir.AluOpType.add,
                        )
                        nc.sync.dma_start(
                            out=out[r0 : r0 + S_SUB, off : off + sz],
                            in_=osb[:, off : off + sz],
                        )
                s0 += sc

    nc.compile()
    return nc


def _get_nc():
    key = "full"
    if key not in _cache:
        _cache[key] = build_nc()
    return _cache[key]


def kernel(x, weight_int8, scale, bias):
    global LAST_RESULT
    x = np.asarray(x, dtype=np.float32)
    w = np.asarray(weight_int8)
    scale_f = np.float32(np.asarray(scale).reshape(()))
    bias = np.asarray(bias, dtype=np.float32)

    # host-side layout prep (sharding): contraction dim to the front; the
    # int8-valued weight is shipped in its compressed (int8) form
    xt = np.ascontiguousarray(x.reshape(S_TOT, IN_F).T)  # [in, s]
    wt_full = np.ascontiguousarray(w.T.astype(np.int8))  # [in, out]
    scale_rep = np.full((1, 1), scale_f, dtype=np.float32)

    nc = _get_nc()
    in_maps = []
    for c in range(NCORES):
        o0, o1 = c * OUT_PER, (c + 1) * OUT_PER
        in_maps.append(
            {
                "xt": xt,
                "wt": np.ascontiguousarray(wt_full[:, o0:o1]),
                "bias": np.ascontiguousarray(bias[o0:o1][None, :]),
                "scale": scale_rep,
            }
        )

    res = run_bass_kernel_spmd(
        nc, in_maps, core_ids=list(range(NCORES)), trace=TRACE
    )
    LAST_RESULT = res
    out = np.concatenate([res.results[c]["out"] for c in range(NCORES)], axis=1)
    return out.reshape(B, S, OUT_F)



# revision 2
# speedup vs baseline: 1.0495x; 1.0495x over previous
"""CompressedLinear Trainium2 kernel — one-level Strassen variant.

out[b,s,o] = x[b,s,i] @ (int8_w[o,i] * scale).T + bias[o]
x: [4,2048,4096] f32, w: [11008,4096] int32 (int8 vals), scale f32, bias [11008].

Sharding: column-parallel over 8 cores (1376 out-features each), x replicated.

Per core, one Strassen level over [S=8192, K=4096] @ [K, O=1376]:
  S split: S1 = rows [0,4096), S2 = [4096,8192)
  K split: K1 = [0,2048), K2 = [2048,4096)
  O split: O1 = [0,688), O2 = [688,1376)
  7 products Mi = Ai @ Bi with [4096 x 2048] @ [2048 x 688]  (7/8 the MACs)
    A1=X11+X22 B1=W11+W22 | A2=X21+X22 B2=W11 | A3=X11 B3=W12-W22
    A4=X22 B4=W21-W11     | A5=X11+X12 B5=W22 | A6=X21-X11 B6=W11+W12
    A7=X12-X22 B7=W21+W22
  O11=M1+M4-M5+M7  O12=M3+M5  O21=M2+M4  O22=M1-M2+M3+M6
All A/B combos are computed on host. B combos are integers |.|<=254 — exact
in bf16. Device: 7 psum banks accumulate the Mi per (s-block, o-chunk);
DVE combines + scale/bias epilogue; DMA stores the four output quadrants.
"""

import numpy as np
import ml_dtypes

import concourse.bacc as bacc
import concourse.mybir as mybir
import concourse.tile as tile
from concourse.bass_utils import run_bass_kernel_spmd

B, S, IN_F, OUT_F = 4, 2048, 4096, 11008
NCORES = 8
OUT_PER = OUT_F // NCORES  # 1376
S_TOT = B * S  # 8192

SV = S_TOT // 2  # 4096 virtual rows
KH = IN_F // 2  # 2048
OH = OUT_PER // 2  # 688
KT = KH // 128  # 16 k-tiles
NVSB = SV // 128  # 32 virtual s-blocks
CPS = [(0, 512), (512, 176)]  # o-chunks within a 688-wide half

bf16np = ml_dtypes.bfloat16

TRACE = False
LAST_RESULT = None

_cache = {}


def build_nc():
    f32 = mybir.dt.float32
    bf16 = mybir.dt.bfloat16

    nc = bacc.Bacc("TRN2", target_bir_lowering=False, debug=False, num_devices=NCORES)

    a_d = [
        nc.dram_tensor(f"a{i}", [KH, SV], bf16, kind="ExternalInput").ap()
        for i in range(7)
    ]
    b_d = [
        nc.dram_tensor(f"b{i}", [KH, OH], bf16, kind="ExternalInput").ap()
        for i in range(7)
    ]
    bias = nc.dram_tensor("bias", [1, OUT_PER], f32, kind="ExternalInput").ap()
    scale = nc.dram_tensor("scale", [1, 1], f32, kind="ExternalInput").ap()
    out = nc.dram_tensor("out", [S_TOT, OUT_PER], f32, kind="ExternalOutput").ap()

    with tile.TileContext(nc) as tc:
        with (
            tc.tile_pool(name="wt", bufs=1) as wt_pool,
            tc.tile_pool(name="abf", bufs=8) as a_pool,
            tc.tile_pool(name="psum", bufs=1, space="PSUM") as psum_pool,
            tc.tile_pool(name="osb", bufs=2) as osb_pool,
            tc.tile_pool(name="consts", bufs=1) as const_pool,
        ):
            # HAM warmup: PE clock-gate ramp (4/8 cold -> 8/8 after ~3.4us).
            zeros = const_pool.tile([128, 512], bf16, tag="zeros", name="zeros")
            nc.vector.memset(zeros[:], 0)
            psw = psum_pool.tile([128, 512], f32, tag="warm", name="warm")
            for i in range(16):
                nc.tensor.matmul(
                    psw[:, :], zeros[:, 0:128], zeros[:, :], start=True, stop=True
                )
            for i in range(44):
                nc.tensor.matmul(
                    psw[:, 0:128], zeros[:, 0:128], zeros[:, 0:128],
                    start=True, stop=True,
                )

            # W' combos: per product, per k-group, per o-chunk-half tiles
            # [128, 4, cpw] bf16. Load order: (h0,g0) -> vsb0's A tiles ->
            # rest of h0 -> h1, so the PE starts real matmuls ~4us in and
            # is never gated on the full 19.7 MB.
            def emit_w(g, h):
                cp0, cpw = CPS[h]
                for i in range(7):
                    t = wt_pool.tile(
                        [128, 4, cpw], bf16, tag=f"w{i}g{g}h{h}", name=f"w{i}g{g}h{h}"
                    )
                    src = b_d[i][
                        g * 512 : (g + 1) * 512, cp0 : cp0 + cpw
                    ].rearrange("(g p) o -> p g o", p=128)
                    nc.sync.dma_start(out=t[:], in_=src)
                    wt_tiles[(i, g, h)] = t

            wt_tiles = {}
            emit_w(0, 0)

            # vsb0 A tiles, hoisted ahead of the remaining W' loads
            a_t_v0 = []
            for i in range(7):
                t = a_pool.tile([128, KT, 128], bf16, tag="a", name=f"a{i}_0")
                src = a_d[i][:, 0:128].rearrange("(g p) s -> p g s", p=128)
                nc.sync.dma_start(out=t[:], in_=src)
                a_t_v0.append(t)

            for g in range(1, 4):
                emit_w(g, 0)
            for g in range(4):
                emit_w(g, 1)

            scale_sb = const_pool.tile([128, 1], f32, tag="scale", name="scale_sb")
            nc.sync.dma_start(out=scale_sb[:], in_=scale.partition_broadcast(128))
            bias_sb = const_pool.tile([128, OUT_PER], f32, tag="bias", name="bias_sb")
            nc.sync.dma_start(out=bias_sb[:], in_=bias.partition_broadcast(128))

            add = mybir.AluOpType.add
            sub = mybir.AluOpType.subtract
            mult = mybir.AluOpType.mult

            for v in range(NVSB):
                s0 = v * 128
                # A tiles for this vsb: [128, 16, 128] bf16 per product
                if v == 0:
                    a_t = a_t_v0
                else:
                    a_t = []
                    for i in range(7):
                        t = a_pool.tile([128, KT, 128], bf16, tag="a", name=f"a{i}_{v}")
                        src = a_d[i][:, s0 : s0 + 128].rearrange(
                            "(g p) s -> p g s", p=128
                        )
                        nc.sync.dma_start(out=t[:], in_=src)
                        a_t.append(t)

                for h, (cp0, cpw) in enumerate(CPS):
                    ps = [
                        psum_pool.tile([128, 512], f32, tag=f"m{i}", name=f"m{i}_{v}_{cp0}")
                        for i in range(7)
                    ]
                    # vsb0: k-group-major so first MMs chase the W' load
                    # stream. Later vsbs: product-major so each A tile's
                    # last read lands early, widening the prefetch window.
                    if v == 0:
                        order = [
                            (i, k)
                            for g in range(4)
                            for i in range(7)
                            for k in range(g * 4, (g + 1) * 4)
                        ]
                    else:
                        order = [(i, k) for i in range(7) for k in range(KT)]
                    for i, k in order:
                        g = k // 4
                        wg = wt_tiles[(i, g, h)]
                        nc.tensor.matmul(
                            ps[i][:, :cpw],
                            a_t[i][:, k, :],
                            wg[:, k - g * 4, :cpw],
                            start=(k == 0),
                            stop=(k == KT - 1),
                        )

                    # combines + epilogue; column ranges: O1-half = cp0,
                    # O2-half = 688+cp0. Row ranges: S1 = s0, S2 = 4096+s0.
                    c1 = cp0
                    c2 = OH + cp0

                    # DVE reads at most one PSUM operand per op: copy first,
                    # then chain single-psum adds, then scale+bias.
                    o21 = osb_pool.tile([128, 512], f32, tag="o21", name=f"o21_{v}_{cp0}")
                    nc.vector.tensor_copy(o21[:, :cpw], ps[1][:, :cpw])
                    nc.vector.tensor_tensor(o21[:, :cpw], o21[:, :cpw], ps[3][:, :cpw], add)
                    nc.vector.scalar_tensor_tensor(
                        o21[:, :cpw], o21[:, :cpw], scale_sb[:, 0:1],
                        bias_sb[:, c1 : c1 + cpw], mult, add,
                    )
                    nc.sync.dma_start(
                        out=out[SV + s0 : SV + s0 + 128, c1 : c1 + cpw], in_=o21[:, :cpw]
                    )

                    o12 = osb_pool.tile([128, 512], f32, tag="o12", name=f"o12_{v}_{cp0}")
                    nc.vector.tensor_copy(o12[:, :cpw], ps[2][:, :cpw])
                    nc.vector.tensor_tensor(o12[:, :cpw], o12[:, :cpw], ps[4][:, :cpw], add)
                    nc.vector.scalar_tensor_tensor(
                        o12[:, :cpw], o12[:, :cpw], scale_sb[:, 0:1],
                        bias_sb[:, c2 : c2 + cpw], mult, add,
                    )
                    nc.sync.dma_start(
                        out=out[s0 : s0 + 128, c2 : c2 + cpw], in_=o12[:, :cpw]
                    )

                    o11 = osb_pool.tile([128, 512], f32, tag="o11", name=f"o11_{v}_{cp0}")
                    nc.vector.tensor_copy(o11[:, :cpw], ps[0][:, :cpw])
                    nc.vector.tensor_tensor(o11[:, :cpw], o11[:, :cpw], ps[3][:, :cpw], add)
                    nc.vector.tensor_tensor(o11[:, :cpw], o11[:, :cpw], ps[4][:, :cpw], sub)
                    nc.vector.tensor_tensor(o11[:, :cpw], o11[:, :cpw], ps[6][:, :cpw], add)
                    nc.vector.scalar_tensor_tensor(
                        o11[:, :cpw], o11[:, :cpw], scale_sb[:, 0:1],
                        bias_sb[:, c1 : c1 + cpw], mult, add,
                    )
                    nc.sync.dma_start(
                        out=out[s0 : s0 + 128, c1 : c1 + cpw], in_=o11[:, :cpw]
                    )

                    o22 = osb_pool.tile([128, 512], f32, tag="o22", name=f"o22_{v}_{cp0}")
                    nc.vector.tensor_copy(o22[:, :cpw], ps[0][:, :cpw])
                    nc.vector.tensor_tensor(o22[:, :cpw], o22[:, :cpw], ps[1][:, :cpw], sub)
                    nc.vector.tensor_tensor(o22[:, :cpw], o22[:, :cpw], ps[2][:, :cpw], add)
                    nc.vector.tensor_tensor(o22[:, :cpw], o22[:, :cpw], ps[5][:, :cpw], add)
                    nc.vector.scalar_tensor_tensor(
                        o22[:, :cpw], o22[:, :cpw], scale_sb[:, 0:1],
                        bias_sb[:, c2 : c2 + cpw], mult, add,
                    )
                    nc.sync.dma_start(
                        out=out[SV + s0 : SV + s0 + 128, c2 : c2 + cpw], in_=o22[:, :cpw]
                    )

    nc.compile()
    return nc


def _get_nc():
    if "s" not in _cache:
        _cache["s"] = build_nc()
    return _cache["s"]


def kernel(x, weight_int8, scale, bias):
    global LAST_RESULT
    x = np.asarray(x, dtype=np.float32)
    w = np.asarray(weight_int8)
    scale_f = np.float32(np.asarray(scale).reshape(()))
    bias = np.asarray(bias, dtype=np.float32)

    xf = x.reshape(S_TOT, IN_F)
    X11 = xf[:SV, :KH]
    X12 = xf[:SV, KH:]
    X21 = xf[SV:, :KH]
    X22 = xf[SV:, KH:]
    a_list = [
        X11 + X22, X21 + X22, X11, X22, X11 + X12, X21 - X11, X12 - X22,
    ]
    # [k, s] bf16, contiguous
    a_np = {
        f"a{i}": np.ascontiguousarray(a.T).astype(bf16np) for i, a in enumerate(a_list)
    }

    wf = w.astype(np.float32)  # [out, in]
    scale_rep = np.full((1, 1), scale_f, dtype=np.float32)

    nc = _get_nc()
    in_maps = []
    for c in range(NCORES):
        o0 = c * OUT_PER
        wc = wf[o0 : o0 + OUT_PER, :].T  # [in, out_per]
        W11 = wc[:KH, :OH]
        W12 = wc[:KH, OH:]
        W21 = wc[KH:, :OH]
        W22 = wc[KH:, OH:]
        b_list = [
            W11 + W22, W11, W12 - W22, W21 - W11, W22, W11 + W12, W21 + W22,
        ]
        m = {
            f"b{i}": np.ascontiguousarray(b).astype(bf16np)
            for i, b in enumerate(b_list)
        }
        m.update(a_np)
        m["bias"] = np.ascontiguousarray(bias[o0 : o0 + OUT_PER][None, :])
        m["scale"] = scale_rep
        in_maps.append(m)

    res = run_bass_kernel_spmd(nc, in_maps, core_ids=list(range(NCORES)), trace=TRACE)
    LAST_RESULT = res
    out = np.concatenate([res.results[c]["out"] for c in range(NCORES)], axis=1)
    return out.reshape(B, S, OUT_F)


# revision 3
# speedup vs baseline: 1.0524x; 1.0028x over previous
"""CompressedLinear Trainium2 kernel — one-level Strassen variant.

out[b,s,o] = x[b,s,i] @ (int8_w[o,i] * scale).T + bias[o]
x: [4,2048,4096] f32, w: [11008,4096] int32 (int8 vals), scale f32, bias [11008].

Sharding: column-parallel over 8 cores (1376 out-features each), x replicated.

Per core, one Strassen level over [S=8192, K=4096] @ [K, O=1376]:
  S split: S1 = rows [0,4096), S2 = [4096,8192)
  K split: K1 = [0,2048), K2 = [2048,4096)
  O split: O1 = [0,688), O2 = [688,1376)
  7 products Mi = Ai @ Bi with [4096 x 2048] @ [2048 x 688]  (7/8 the MACs)
    A1=X11+X22 B1=W11+W22 | A2=X21+X22 B2=W11 | A3=X11 B3=W12-W22
    A4=X22 B4=W21-W11     | A5=X11+X12 B5=W22 | A6=X21-X11 B6=W11+W12
    A7=X12-X22 B7=W21+W22
  O11=M1+M4-M5+M7  O12=M3+M5  O21=M2+M4  O22=M1-M2+M3+M6
All A/B combos are computed on host. B combos are integers |.|<=254 — exact
in bf16. Device: 7 psum banks accumulate the Mi per (s-block, o-chunk);
DVE combines + scale/bias epilogue; DMA stores the four output quadrants.
"""

import numpy as np
import ml_dtypes

import concourse.bacc as bacc
import concourse.mybir as mybir
import concourse.tile as tile
from concourse.bass_utils import run_bass_kernel_spmd

B, S, IN_F, OUT_F = 4, 2048, 4096, 11008
NCORES = 8
OUT_PER = OUT_F // NCORES  # 1376
S_TOT = B * S  # 8192

SV = S_TOT // 2  # 4096 virtual rows
KH = IN_F // 2  # 2048
OH = OUT_PER // 2  # 688
KT = KH // 128  # 16 k-tiles
NVSB = SV // 128  # 32 virtual s-blocks
CPS = [(0, 512), (512, 176)]  # o-chunks within a 688-wide half

bf16np = ml_dtypes.bfloat16

TRACE = False
LAST_RESULT = None

_cache = {}


def build_nc():
    f32 = mybir.dt.float32
    bf16 = mybir.dt.bfloat16

    nc = bacc.Bacc("TRN2", target_bir_lowering=False, debug=False, num_devices=NCORES)

    a_d = [
        nc.dram_tensor(f"a{i}", [KH, SV], bf16, kind="ExternalInput").ap()
        for i in range(7)
    ]
    b_d = [
        nc.dram_tensor(f"b{i}", [KH, OH], bf16, kind="ExternalInput").ap()
        for i in range(7)
    ]
    bias = nc.dram_tensor("bias", [1, OUT_PER], f32, kind="ExternalInput").ap()
    scale = nc.dram_tensor("scale", [1, 1], f32, kind="ExternalInput").ap()
    out = nc.dram_tensor("out", [S_TOT, OUT_PER], f32, kind="ExternalOutput").ap()

    with tile.TileContext(nc) as tc:
        with (
            tc.tile_pool(name="wt", bufs=1) as wt_pool,
            tc.tile_pool(name="abf", bufs=8) as a_pool,
            tc.tile_pool(name="psum", bufs=1, space="PSUM") as psum_pool,
            tc.tile_pool(name="osb", bufs=2) as osb_pool,
            tc.tile_pool(name="consts", bufs=1) as const_pool,
        ):
            # HAM warmup: PE clock-gate ramp (4/8 cold -> 8/8 after ~3.4us).
            zeros = const_pool.tile([128, 512], bf16, tag="zeros", name="zeros")
            nc.vector.memset(zeros[:], 0)
            psw = psum_pool.tile([128, 512], f32, tag="warm", name="warm")
            for i in range(16):
                nc.tensor.matmul(
                    psw[:, :], zeros[:, 0:128], zeros[:, :], start=True, stop=True
                )
            for i in range(44):
                nc.tensor.matmul(
                    psw[:, 0:128], zeros[:, 0:128], zeros[:, 0:128],
                    start=True, stop=True,
                )

            # W' combos: per product, per k-group, per o-chunk-half tiles
            # [128, 4, cpw] bf16. Load order: (h0,g0) -> vsb0's A tiles ->
            # rest of h0 -> h1, so the PE starts real matmuls ~4us in and
            # is never gated on the full 19.7 MB.
            def emit_w(g, h):
                cp0, cpw = CPS[h]
                for i in range(7):
                    t = wt_pool.tile(
                        [128, 4, cpw], bf16, tag=f"w{i}g{g}h{h}", name=f"w{i}g{g}h{h}"
                    )
                    src = b_d[i][
                        g * 512 : (g + 1) * 512, cp0 : cp0 + cpw
                    ].rearrange("(g p) o -> p g o", p=128)
                    nc.sync.dma_start(out=t[:], in_=src)
                    wt_tiles[(i, g, h)] = t

            wt_tiles = {}
            emit_w(0, 0)

            # vsb0 A tiles, hoisted ahead of the remaining W' loads
            a_t_v0 = []
            for i in range(7):
                t = a_pool.tile([128, KT, 128], bf16, tag="a", name=f"a{i}_0")
                src = a_d[i][:, 0:128].rearrange("(g p) s -> p g s", p=128)
                nc.sync.dma_start(out=t[:], in_=src)
                a_t_v0.append(t)

            for g in range(1, 4):
                emit_w(g, 0)
            for g in range(4):
                emit_w(g, 1)

            scale_sb = const_pool.tile([128, 1], f32, tag="scale", name="scale_sb")
            nc.sync.dma_start(out=scale_sb[:], in_=scale.partition_broadcast(128))
            bias_sb = const_pool.tile([128, OUT_PER], f32, tag="bias", name="bias_sb")
            nc.sync.dma_start(out=bias_sb[:], in_=bias.partition_broadcast(128))

            add = mybir.AluOpType.add
            sub = mybir.AluOpType.subtract
            mult = mybir.AluOpType.mult

            for v in range(NVSB):
                s0 = v * 128
                # A tiles for this vsb: [128, 16, 128] bf16 per product
                if v == 0:
                    a_t = a_t_v0
                else:
                    a_t = []
                    for i in range(7):
                        t = a_pool.tile([128, KT, 128], bf16, tag="a", name=f"a{i}_{v}")
                        src = a_d[i][:, s0 : s0 + 128].rearrange(
                            "(g p) s -> p g s", p=128
                        )
                        nc.sync.dma_start(out=t[:], in_=src)
                        a_t.append(t)

                for h, (cp0, cpw) in enumerate(CPS):
                    ps = [
                        psum_pool.tile([128, 512], f32, tag=f"m{i}", name=f"m{i}_{v}_{cp0}")
                        for i in range(7)
                    ]
                    # vsb0: k-group-major so first MMs chase the W' load
                    # stream. Later vsbs: product-major so each A tile's
                    # last read lands early, widening the prefetch window.
                    if v == 0:
                        order = [
                            (i, k)
                            for g in range(4)
                            for i in range(7)
                            for k in range(g * 4, (g + 1) * 4)
                        ]
                    else:
                        order = [(i, k) for i in range(7) for k in range(KT)]
                    # during the W'-load chase (first vsbs), pad each
                    # 4-MM run with short no-dep filler MMs: the in-order PE
                    # queue executes them where it would otherwise idle on
                    # DMA, keeping the clock ramp (P-state) from resetting.
                    filler = {0: 3, 1: 2, 2: 1}.get(v, 0)
                    for n_mm, (i, k) in enumerate(order):
                        g = k // 4
                        wg = wt_tiles[(i, g, h)]
                        nc.tensor.matmul(
                            ps[i][:, :cpw],
                            a_t[i][:, k, :],
                            wg[:, k - g * 4, :cpw],
                            start=(k == 0),
                            stop=(k == KT - 1),
                        )
                        if filler and n_mm % 4 == 3:
                            for _ in range(filler):
                                nc.tensor.matmul(
                                    psw[:, 0:128], zeros[:, 0:128],
                                    zeros[:, 0:128], start=True, stop=True,
                                )

                    # combines + epilogue; column ranges: O1-half = cp0,
                    # O2-half = 688+cp0. Row ranges: S1 = s0, S2 = 4096+s0.
                    c1 = cp0
                    c2 = OH + cp0

                    # DVE reads at most one PSUM operand per op: copy first,
                    # then chain single-psum adds, then scale+bias.
                    o21 = osb_pool.tile([128, 512], f32, tag="o21", name=f"o21_{v}_{cp0}")
                    nc.vector.tensor_copy(o21[:, :cpw], ps[1][:, :cpw])
                    nc.vector.tensor_tensor(o21[:, :cpw], o21[:, :cpw], ps[3][:, :cpw], add)
                    nc.vector.scalar_tensor_tensor(
                        o21[:, :cpw], o21[:, :cpw], scale_sb[:, 0:1],
                        bias_sb[:, c1 : c1 + cpw], mult, add,
                    )
                    nc.sync.dma_start(
                        out=out[SV + s0 : SV + s0 + 128, c1 : c1 + cpw], in_=o21[:, :cpw]
                    )

                    o12 = osb_pool.tile([128, 512], f32, tag="o12", name=f"o12_{v}_{cp0}")
                    nc.vector.tensor_copy(o12[:, :cpw], ps[2][:, :cpw])
                    nc.vector.tensor_tensor(o12[:, :cpw], o12[:, :cpw], ps[4][:, :cpw], add)
                    nc.vector.scalar_tensor_tensor(
                        o12[:, :cpw], o12[:, :cpw], scale_sb[:, 0:1],
                        bias_sb[:, c2 : c2 + cpw], mult, add,
                    )
                    nc.sync.dma_start(
                        out=out[s0 : s0 + 128, c2 : c2 + cpw], in_=o12[:, :cpw]
                    )

                    o11 = osb_pool.tile([128, 512], f32, tag="o11", name=f"o11_{v}_{cp0}")
                    nc.vector.tensor_copy(o11[:, :cpw], ps[0][:, :cpw])
                    nc.vector.tensor_tensor(o11[:, :cpw], o11[:, :cpw], ps[3][:, :cpw], add)
                    nc.vector.tensor_tensor(o11[:, :cpw], o11[:, :cpw], ps[4][:, :cpw], sub)
                    nc.vector.tensor_tensor(o11[:, :cpw], o11[:, :cpw], ps[6][:, :cpw], add)
                    nc.vector.scalar_tensor_tensor(
                        o11[:, :cpw], o11[:, :cpw], scale_sb[:, 0:1],
                        bias_sb[:, c1 : c1 + cpw], mult, add,
                    )
                    nc.sync.dma_start(
                        out=out[s0 : s0 + 128, c1 : c1 + cpw], in_=o11[:, :cpw]
                    )

                    o22 = osb_pool.tile([128, 512], f32, tag="o22", name=f"o22_{v}_{cp0}")
                    nc.vector.tensor_copy(o22[:, :cpw], ps[0][:, :cpw])
                    nc.vector.tensor_tensor(o22[:, :cpw], o22[:, :cpw], ps[1][:, :cpw], sub)
                    nc.vector.tensor_tensor(o22[:, :cpw], o22[:, :cpw], ps[2][:, :cpw], add)
                    nc.vector.tensor_tensor(o22[:, :cpw], o22[:, :cpw], ps[5][:, :cpw], add)
                    nc.vector.scalar_tensor_tensor(
                        o22[:, :cpw], o22[:, :cpw], scale_sb[:, 0:1],
                        bias_sb[:, c2 : c2 + cpw], mult, add,
                    )
                    nc.sync.dma_start(
                        out=out[SV + s0 : SV + s0 + 128, c2 : c2 + cpw], in_=o22[:, :cpw]
                    )

    nc.compile()
    return nc


def _get_nc():
    if "s" not in _cache:
        _cache["s"] = build_nc()
    return _cache["s"]


def kernel(x, weight_int8, scale, bias):
    global LAST_RESULT
    x = np.asarray(x, dtype=np.float32)
    w = np.asarray(weight_int8)
    scale_f = np.float32(np.asarray(scale).reshape(()))
    bias = np.asarray(bias, dtype=np.float32)

    xf = x.reshape(S_TOT, IN_F)
    X11 = xf[:SV, :KH]
    X12 = xf[:SV, KH:]
    X21 = xf[SV:, :KH]
    X22 = xf[SV:, KH:]
    a_list = [
        X11 + X22, X21 + X22, X11, X22, X11 + X12, X21 - X11, X12 - X22,
    ]
    # [k, s] bf16, contiguous
    a_np = {
        f"a{i}": np.ascontiguousarray(a.T).astype(bf16np) for i, a in enumerate(a_list)
    }

    wf = w.astype(np.float32)  # [out, in]
    scale_rep = np.full((1, 1), scale_f, dtype=np.float32)

    nc = _get_nc()
    in_maps = []
    for c in range(NCORES):
        o0 = c * OUT_PER
        wc = wf[o0 : o0 + OUT_PER, :].T  # [in, out_per]
        W11 = wc[:KH, :OH]
        W12 = wc[:KH, OH:]
        W21 = wc[KH:, :OH]
        W22 = wc[KH:, OH:]
        b_list = [
            W11 + W22, W11, W12 - W22, W21 - W11, W22, W11 + W12, W21 + W22,
        ]
        m = {
            f"b{i}": np.ascontiguousarray(b).astype(bf16np)
            for i, b in enumerate(b_list)
        }
        m.update(a_np)
        m["bias"] = np.ascontiguousarray(bias[o0 : o0 + OUT_PER][None, :])
        m["scale"] = scale_rep
        in_maps.append(m)

    res = run_bass_kernel_spmd(nc, in_maps, core_ids=list(range(NCORES)), trace=TRACE)
    LAST_RESULT = res
    out = np.concatenate([res.results[c]["out"] for c in range(NCORES)], axis=1)
    return out.reshape(B, S, OUT_F)


# revision 4
# speedup vs baseline: 1.0610x; 1.0081x over previous
"""CompressedLinear Trainium2 kernel — one-level Strassen variant.

out[b,s,o] = x[b,s,i] @ (int8_w[o,i] * scale).T + bias[o]
x: [4,2048,4096] f32, w: [11008,4096] int32 (int8 vals), scale f32, bias [11008].

Sharding: column-parallel over 8 cores (1376 out-features each), x replicated.

Per core, one Strassen level over [S=8192, K=4096] @ [K, O=1376]:
  S split: S1 = rows [0,4096), S2 = [4096,8192)
  K split: K1 = [0,2048), K2 = [2048,4096)
  O split: O1 = [0,688), O2 = [688,1376)
  7 products Mi = Ai @ Bi with [4096 x 2048] @ [2048 x 688]  (7/8 the MACs)
    A1=X11+X22 B1=W11+W22 | A2=X21+X22 B2=W11 | A3=X11 B3=W12-W22
    A4=X22 B4=W21-W11     | A5=X11+X12 B5=W22 | A6=X21-X11 B6=W11+W12
    A7=X12-X22 B7=W21+W22
  O11=M1+M4-M5+M7  O12=M3+M5  O21=M2+M4  O22=M1-M2+M3+M6
All A/B combos are computed on host. B combos are integers |.|<=254 — exact
in bf16. Device: 7 psum banks accumulate the Mi per (s-block, o-chunk);
DVE combines + scale/bias epilogue; DMA stores the four output quadrants.
"""

import numpy as np
import ml_dtypes

import concourse.bacc as bacc
import concourse.mybir as mybir
import concourse.tile as tile
from concourse.bass_utils import run_bass_kernel_spmd

B, S, IN_F, OUT_F = 4, 2048, 4096, 11008
NCORES = 8
OUT_PER = OUT_F // NCORES  # 1376
S_TOT = B * S  # 8192

SV = S_TOT // 2  # 4096 virtual rows
KH = IN_F // 2  # 2048
OH = OUT_PER // 2  # 688
KT = KH // 128  # 16 k-tiles
NVSB = SV // 128  # 32 virtual s-blocks
CPS = [(0, 512), (512, 176)]  # o-chunks within a 688-wide half

bf16np = ml_dtypes.bfloat16

TRACE = False
LAST_RESULT = None

_cache = {}


def build_nc():
    f32 = mybir.dt.float32
    bf16 = mybir.dt.bfloat16

    nc = bacc.Bacc("TRN2", target_bir_lowering=False, debug=False, num_devices=NCORES)

    a_d = [
        nc.dram_tensor(f"a{i}", [KH, SV], bf16, kind="ExternalInput").ap()
        for i in range(7)
    ]
    b_d = {
        (i, h): nc.dram_tensor(
            f"b{i}h{h}", [4 * 128, 4 * CPS[h][1]], bf16, kind="ExternalInput"
        ).ap()
        for i in range(7)
        for h in range(2)
    }
    bias = nc.dram_tensor("bias", [1, OUT_PER], f32, kind="ExternalInput").ap()
    scale = nc.dram_tensor("scale", [1, 1], f32, kind="ExternalInput").ap()
    out = nc.dram_tensor("out", [S_TOT, OUT_PER], f32, kind="ExternalOutput").ap()

    with tile.TileContext(nc) as tc:
        with (
            tc.tile_pool(name="wt", bufs=1) as wt_pool,
            tc.tile_pool(name="abf", bufs=8) as a_pool,
            tc.tile_pool(name="psum", bufs=1, space="PSUM") as psum_pool,
            tc.tile_pool(name="osb", bufs=2) as osb_pool,
            tc.tile_pool(name="consts", bufs=1) as const_pool,
        ):
            # HAM warmup: PE clock-gate ramp (4/8 cold -> 8/8 after ~3.4us).
            zeros = const_pool.tile([128, 512], bf16, tag="zeros", name="zeros")
            nc.vector.memset(zeros[:], 0)
            psw = psum_pool.tile([128, 512], f32, tag="warm", name="warm")
            for i in range(16):
                nc.tensor.matmul(
                    psw[:, :], zeros[:, 0:128], zeros[:, :], start=True, stop=True
                )
            for i in range(44):
                nc.tensor.matmul(
                    psw[:, 0:128], zeros[:, 0:128], zeros[:, 0:128],
                    start=True, stop=True,
                )

            # W' combos: per product, per k-group, per o-chunk-half tiles
            # [128, 4, cpw] bf16. Load order: (h0,g0) -> vsb0's A tiles ->
            # rest of h0 -> h1, so the PE starts real matmuls ~4us in and
            # is never gated on the full 19.7 MB.
            def emit_w(g, h):
                cp0, cpw = CPS[h]
                for i in range(7):
                    t = wt_pool.tile(
                        [128, 4, cpw], bf16, tag=f"w{i}g{g}h{h}", name=f"w{i}g{g}h{h}"
                    )
                    src = b_d[(i, h)][g * 128 : (g + 1) * 128, :].rearrange(
                        "p (g o) -> p g o", g=4
                    )
                    nc.sync.dma_start(out=t[:], in_=src)
                    wt_tiles[(i, g, h)] = t

            wt_tiles = {}
            emit_w(0, 0)

            # vsb0 A tiles, hoisted ahead of the remaining W' loads
            a_t_v0 = []
            for i in range(7):
                t = a_pool.tile([128, KT, 128], bf16, tag="a", name=f"a{i}_0")
                src = a_d[i][:, 0:128].rearrange("(g p) s -> p g s", p=128)
                nc.sync.dma_start(out=t[:], in_=src)
                a_t_v0.append(t)

            for g in range(1, 4):
                emit_w(g, 0)
            for g in range(4):
                emit_w(g, 1)

            scale_sb = const_pool.tile([128, 1], f32, tag="scale", name="scale_sb")
            nc.sync.dma_start(out=scale_sb[:], in_=scale.partition_broadcast(128))
            bias_sb = const_pool.tile([128, OUT_PER], f32, tag="bias", name="bias_sb")
            nc.sync.dma_start(out=bias_sb[:], in_=bias.partition_broadcast(128))

            add = mybir.AluOpType.add
            sub = mybir.AluOpType.subtract
            mult = mybir.AluOpType.mult

            for v in range(NVSB):
                s0 = v * 128
                # A tiles for this vsb: [128, 16, 128] bf16 per product
                if v == 0:
                    a_t = a_t_v0
                else:
                    a_t = []
                    for i in range(7):
                        t = a_pool.tile([128, KT, 128], bf16, tag="a", name=f"a{i}_{v}")
                        src = a_d[i][:, s0 : s0 + 128].rearrange(
                            "(g p) s -> p g s", p=128
                        )
                        nc.sync.dma_start(out=t[:], in_=src)
                        a_t.append(t)

                for h, (cp0, cpw) in enumerate(CPS):
                    ps = [
                        psum_pool.tile([128, 512], f32, tag=f"m{i}", name=f"m{i}_{v}_{cp0}")
                        for i in range(7)
                    ]
                    # vsb0: k-group-major so first MMs chase the W' load
                    # stream. Later vsbs: product-major so each A tile's
                    # last read lands early, widening the prefetch window.
                    if v == 0:
                        order = [
                            (i, k)
                            for g in range(4)
                            for i in range(7)
                            for k in range(g * 4, (g + 1) * 4)
                        ]
                    else:
                        order = [(i, k) for i in range(7) for k in range(KT)]
                    # during the W'-load chase (first vsbs), pad each
                    # 4-MM run with short no-dep filler MMs: the in-order PE
                    # queue executes them where it would otherwise idle on
                    # DMA, keeping the clock ramp (P-state) from resetting.
                    filler = {0: 3, 1: 2, 2: 1}.get(v, 0)
                    for n_mm, (i, k) in enumerate(order):
                        g = k // 4
                        wg = wt_tiles[(i, g, h)]
                        nc.tensor.matmul(
                            ps[i][:, :cpw],
                            a_t[i][:, k, :],
                            wg[:, k - g * 4, :cpw],
                            start=(k == 0),
                            stop=(k == KT - 1),
                        )
                        if filler and n_mm % 4 == 3:
                            for _ in range(filler):
                                nc.tensor.matmul(
                                    psw[:, 0:128], zeros[:, 0:128],
                                    zeros[:, 0:128], start=True, stop=True,
                                )

                    # combines + epilogue; column ranges: O1-half = cp0,
                    # O2-half = 688+cp0. Row ranges: S1 = s0, S2 = 4096+s0.
                    c1 = cp0
                    c2 = OH + cp0

                    # DVE reads at most one PSUM operand per op: copy first,
                    # then chain single-psum adds, then scale+bias.
                    o21 = osb_pool.tile([128, 512], f32, tag="o21", name=f"o21_{v}_{cp0}")
                    nc.vector.tensor_copy(o21[:, :cpw], ps[1][:, :cpw])
                    nc.vector.tensor_tensor(o21[:, :cpw], o21[:, :cpw], ps[3][:, :cpw], add)
                    nc.vector.scalar_tensor_tensor(
                        o21[:, :cpw], o21[:, :cpw], scale_sb[:, 0:1],
                        bias_sb[:, c1 : c1 + cpw], mult, add,
                    )
                    nc.sync.dma_start(
                        out=out[SV + s0 : SV + s0 + 128, c1 : c1 + cpw], in_=o21[:, :cpw]
                    )

                    o12 = osb_pool.tile([128, 512], f32, tag="o12", name=f"o12_{v}_{cp0}")
                    nc.vector.tensor_copy(o12[:, :cpw], ps[2][:, :cpw])
                    nc.vector.tensor_tensor(o12[:, :cpw], o12[:, :cpw], ps[4][:, :cpw], add)
                    nc.vector.scalar_tensor_tensor(
                        o12[:, :cpw], o12[:, :cpw], scale_sb[:, 0:1],
                        bias_sb[:, c2 : c2 + cpw], mult, add,
                    )
                    nc.sync.dma_start(
                        out=out[s0 : s0 + 128, c2 : c2 + cpw], in_=o12[:, :cpw]
                    )

                    o11 = osb_pool.tile([128, 512], f32, tag="o11", name=f"o11_{v}_{cp0}")
                    nc.vector.tensor_copy(o11[:, :cpw], ps[0][:, :cpw])
                    nc.vector.tensor_tensor(o11[:, :cpw], o11[:, :cpw], ps[3][:, :cpw], add)
                    nc.vector.tensor_tensor(o11[:, :cpw], o11[:, :cpw], ps[4][:, :cpw], sub)
                    nc.vector.tensor_tensor(o11[:, :cpw], o11[:, :cpw], ps[6][:, :cpw], add)
                    nc.vector.scalar_tensor_tensor(
                        o11[:, :cpw], o11[:, :cpw], scale_sb[:, 0:1],
                        bias_sb[:, c1 : c1 + cpw], mult, add,
                    )
                    nc.sync.dma_start(
                        out=out[s0 : s0 + 128, c1 : c1 + cpw], in_=o11[:, :cpw]
                    )

                    o22 = osb_pool.tile([128, 512], f32, tag="o22", name=f"o22_{v}_{cp0}")
                    nc.vector.tensor_copy(o22[:, :cpw], ps[0][:, :cpw])
                    nc.vector.tensor_tensor(o22[:, :cpw], o22[:, :cpw], ps[1][:, :cpw], sub)
                    nc.vector.tensor_tensor(o22[:, :cpw], o22[:, :cpw], ps[2][:, :cpw], add)
                    nc.vector.tensor_tensor(o22[:, :cpw], o22[:, :cpw], ps[5][:, :cpw], add)
                    nc.vector.scalar_tensor_tensor(
                        o22[:, :cpw], o22[:, :cpw], scale_sb[:, 0:1],
                        bias_sb[:, c2 : c2 + cpw], mult, add,
                    )
                    nc.sync.dma_start(
                        out=out[SV + s0 : SV + s0 + 128, c2 : c2 + cpw], in_=o22[:, :cpw]
                    )

    nc.compile()
    return nc


def _get_nc():
    if "s" not in _cache:
        _cache["s"] = build_nc()
    return _cache["s"]


def kernel(x, weight_int8, scale, bias):
    global LAST_RESULT
    x = np.asarray(x, dtype=np.float32)
    w = np.asarray(weight_int8)
    scale_f = np.float32(np.asarray(scale).reshape(()))
    bias = np.asarray(bias, dtype=np.float32)

    xf = x.reshape(S_TOT, IN_F)
    X11 = xf[:SV, :KH]
    X12 = xf[:SV, KH:]
    X21 = xf[SV:, :KH]
    X22 = xf[SV:, KH:]
    a_list = [
        X11 + X22, X21 + X22, X11, X22, X11 + X12, X21 - X11, X12 - X22,
    ]
    # [k, s] bf16, contiguous
    a_np = {
        f"a{i}": np.ascontiguousarray(a.T).astype(bf16np) for i, a in enumerate(a_list)
    }

    wf = w.astype(np.float32)  # [out, in]
    scale_rep = np.full((1, 1), scale_f, dtype=np.float32)

    nc = _get_nc()
    in_maps = []
    for c in range(NCORES):
        o0 = c * OUT_PER
        wc = wf[o0 : o0 + OUT_PER, :].T  # [in, out_per]
        W11 = wc[:KH, :OH]
        W12 = wc[:KH, OH:]
        W21 = wc[KH:, :OH]
        W22 = wc[KH:, OH:]
        b_list = [
            W11 + W22, W11, W12 - W22, W21 - W11, W22, W11 + W12, W21 + W22,
        ]
        m = {}
        for i, b in enumerate(b_list):
            b4 = b.reshape(4, 4, 128, OH)  # [g, gg, p, o]
            for h, (cp0, cpw) in enumerate(CPS):
                arr = b4[:, :, :, cp0 : cp0 + cpw].transpose(0, 2, 1, 3)
                m[f"b{i}h{h}"] = np.ascontiguousarray(
                    arr.reshape(4 * 128, 4 * cpw)
                ).astype(bf16np)
        m.update(a_np)
        m["bias"] = np.ascontiguousarray(bias[o0 : o0 + OUT_PER][None, :])
        m["scale"] = scale_rep
        in_maps.append(m)

    res = run_bass_kernel_spmd(nc, in_maps, core_ids=list(range(NCORES)), trace=TRACE)
    LAST_RESULT = res
    out = np.concatenate([res.results[c]["out"] for c in range(NCORES)], axis=1)
    return out.reshape(B, S, OUT_F)
